# revision 1
# baseline (speedup 1.0000x reference)
"""Trainium2 Bass kernel: 8-head attention block (BN-folded projections,
relative-position bias, softmax, GELU + output projection).

Sharding: data-parallel over batch across 8 NeuronCores (2 batch elems/core).
All weights / bias tables replicated; no collectives.

Per-core layout strategy (all "T" tensors are [channel, position]):
  XT  [c=256, n=1024]  bf16   (host pre-transposed, pre-cast)
  QT/KT [d=256, n]     bf16   rows grouped 32/head -> 4 heads per 128-row tile
  V    [n, 512]        bf16   (natural layout, heads contiguous 64-wide)
  dotsT[j, i] psum f32 = sum_d KT[d,j] QT[d,i]  (4 heads packed via row groups)
                       + bias[j,i]/scale^2      (identity-matmul accumulate)
  exp  = ScalarE Exp(scale * psum) -> bf16
  AV: outT[d, i] = sum_j V[j,d] exp[j,i], head pairs packed via PE col groups
      so pair output fills psum partitions 0..127 = gelu tile layout.
  softmax sums via 64-wide all-ones stationary matmuls in the same col
  groups as AV, so each head's sum is replicated across the same 64
  partitions as its AV rows (no cross-partition moves); reciprocal on DVE,
  one fused mul -> gelu input.
  GELU (ScalarE, +BN_v offset as per-partition bias) -> out proj -> +bias -> DMA.

  HW workarounds (device crashes otherwise, found by probing):
  - tile_position (96,0) is fatal (quadrant-3 bug): head 3 runs as a K=64
    matmul at (64,0) against a KT copy with head-2 rows zeroed.
  - tile_position'd matmuls need PSUM-bank-aligned outputs: every packed
    matmul output is a full 2KB bank.
  - exp and gelu are forced into disjoint program phases so the ScalarE
    activation table loads exactly twice.
"""

import os
import numpy as np
import ml_dtypes

import concourse.bass as bass
import concourse.tile as tile
from concourse import bacc, mybir
from concourse.bass_utils import run_bass_kernel_spmd
from concourse.tile import add_dep_helper

NPBF16 = ml_dtypes.bfloat16
BF16 = mybir.dt.bfloat16
F32 = mybir.dt.float32

HEADS, DK, DV = 8, 32, 64
N = 1024          # positions = 32*32
C = 256           # channels
IDV = HEADS * DV  # 512
NCORES = 8
BLOC = 2          # batch elems per core
SCALE = float(DK) ** -0.5
EPS = 1e-5

_CACHE = {}


def _build_nc():
    nc = bacc.Bacc("TRN2", target_bir_lowering=False, debug=False)

    xt_d = nc.declare_dram_parameter("xt", [BLOC, 2, 128, N], BF16, isOutput=False)
    wq_d = nc.declare_dram_parameter("wq", [128, 2, C], BF16, isOutput=False)
    wk_d = nc.declare_dram_parameter("wk", [128, 2, C], BF16, isOutput=False)
    wv_d = nc.declare_dram_parameter("wv", [128, 2, IDV], BF16, isOutput=False)
    wo_d = nc.declare_dram_parameter("wo", [128, 4, C], BF16, isOutput=False)
    oq_d = nc.declare_dram_parameter("oq", [128, 2], F32, isOutput=False)
    ok_d = nc.declare_dram_parameter("ok", [128, 2], F32, isOutput=False)
    ovg_d = nc.declare_dram_parameter("ovg", [128, 4], F32, isOutput=False)
    bout_d = nc.declare_dram_parameter("bout", [128, C], F32, isOutput=False)
    # bias[hg, is, jt, j1, h, i1] = pos_bias[j, i, 4*hg+h] / SCALE^2
    bias_d = nc.declare_dram_parameter("bias", [2, 2, 8, 128, 4, 512], BF16,
                                       isOutput=False)
    id_d = nc.declare_dram_parameter("ident", [128, 128], BF16, isOutput=False)
    out_d = nc.declare_dram_parameter("out", [BLOC, N, C], F32, isOutput=True)

    Exp = mybir.ActivationFunctionType.Exp
    Gelu = mybir.ActivationFunctionType.Gelu

    with tile.TileContext(nc) as tc:
        with (
            tc.tile_pool(name="const", bufs=1) as const,
            tc.tile_pool(name="persist", bufs=1) as persist,
            tc.tile_pool(name="biasp", bufs=12) as biasp,
            tc.tile_pool(name="expp", bufs=10) as expp,
            tc.tile_pool(name="recp", bufs=3) as recp,
            tc.tile_pool(name="outp", bufs=4) as outp,
            tc.tile_pool(name="dpsum", bufs=3, space="PSUM") as dpsum,
            tc.tile_pool(name="avpsum", bufs=1, space="PSUM") as avpsum,
        ):
            dma = nc.sync

            # ---------------- constants ----------------
            wq_s = const.tile([128, 2, C], BF16, tag="wq")
            dma.dma_start(wq_s[:], wq_d[:])
            wk_s = const.tile([128, 2, C], BF16, tag="wk")
            dma.dma_start(wk_s[:], wk_d[:])
            wv_s = const.tile([128, 2, IDV], BF16, tag="wv")
            dma.dma_start(wv_s[:], wv_d[:])
            wo_s = const.tile([128, 4, C], BF16, tag="wo")
            dma.dma_start(wo_s[:], wo_d[:])
            oq_s = const.tile([128, 2], F32, tag="oq")
            dma.dma_start(oq_s[:], oq_d[:])
            ok_s = const.tile([128, 2], F32, tag="ok")
            dma.dma_start(ok_s[:], ok_d[:])
            ovg_s = const.tile([128, 4], F32, tag="ovg")
            dma.dma_start(ovg_s[:], ovg_d[:])
            bout_s = const.tile([128, C], F32, tag="bout")
            dma.dma_start(bout_s[:], bout_d[:])
            ident_s = const.tile([128, 128], BF16, tag="ident")
            dma.dma_start(ident_s[:], id_d[:])
            ones_s = const.tile([128, 64], BF16, tag="ones")
            nc.vector.memset(ones_s[:], 1.0)

            # ---------------- load x (pre-transposed on host) ----------------
            xt = {}
            for b in range(BLOC):
                for ct in range(2):
                    t = persist.tile([128, N], BF16, tag=f"xt{b}{ct}", name=f"xt{b}{ct}")
                    dma.dma_start(t[:], xt_d[b, ct])
                    xt[b, ct] = t

            # ---------------- Q/K projections -> QT/KT [d, i] bf16 ----------
            qt, kt, kzt = {}, {}, {}
            for b in range(BLOC):
                for tt in range(2):  # 128-row d-tile == head group tt
                    qtile = persist.tile([128, N], BF16, tag=f"qt{b}{tt}", name=f"qt{b}{tt}")
                    ktile = persist.tile([128, N], BF16, tag=f"kt{b}{tt}", name=f"kt{b}{tt}")
                    qt[b, tt], kt[b, tt] = qtile, ktile
                    # kz: copy of KT with head-2 rows zeroed; lets head 3 run
                    # as a K=64 matmul at row group 2 (tile_position (96,0)
                    # crashes this runtime: quadrant-3 HW bug)
                    kz = persist.tile([128, N], BF16, tag=f"kz{b}{tt}",
                                      name=f"kz{b}{tt}")
                    kzt[b, tt] = kz
                    nc.vector.memset(kz[64:96, :], 0.0)
                    for wsb, osb, dst in ((wq_s, oq_s, qtile), (wk_s, ok_s, ktile)):
                        for i2 in range(2):
                            ps = dpsum.tile([128, 512], F32, tag="dps")
                            for ct in range(2):
                                nc.tensor.matmul(
                                    ps[:],
                                    wsb[:, ct, tt * 128:(tt + 1) * 128],
                                    xt[b, ct][:, i2 * 512:(i2 + 1) * 512],
                                    start=(ct == 0), stop=(ct == 1),
                                )
                            nc.vector.tensor_scalar_add(
                                dst[:, i2 * 512:(i2 + 1) * 512], ps[:],
                                osb[:, tt:tt + 1])
                            if dst is ktile:
                                nc.vector.tensor_scalar_add(
                                    kz[96:128, i2 * 512:(i2 + 1) * 512],
                                    ps[96:128, :], osb[96:128, tt:tt + 1])

            # ---------------- V projection -> V [j, (h d)] bf16 --------------
            vt = {}
            for b in range(BLOC):
                for it in range(8):
                    v = persist.tile([128, 8, DV], BF16, tag=f"v{b}{it}", name=f"v{b}{it}")
                    vt[b, it] = v
                    ps = dpsum.tile([128, 512], F32, tag="dps")
                    for ct in range(2):
                        nc.tensor.matmul(
                            ps[:],
                            xt[b, ct][:, it * 128:(it + 1) * 128],
                            wv_s[:, ct, :],
                            start=(ct == 0), stop=(ct == 1),
                        )
                    nc.vector.tensor_copy(v[:, :, :], ps[:].rearrange("p (h d) -> p h d", h=8))

            # gelu input tiles [128 = head pair, 1024] bf16, per (b, dt)
            gelu_t = {}
            for b in range(BLOC):
                for dt in range(4):
                    gelu_t[b, dt] = persist.tile([128, N], BF16, tag=f"g{b}{dt}", name=f"g{b}{dt}")

            last_exp = [None]
            # ---------------- attention ----------------
            for hg in range(2):          # head group of 4 (row-packed dots)
                for isl in range(2):     # i slice of 512
                    bias_t = {}
                    for jt in range(8):
                        bt = biasp.tile([128, 4, 512], BF16, tag="bias", name=f"bias{hg}{isl}{jt}")
                        dma.dma_start(bt[:], bias_d[hg, isl, jt])
                        bias_t[jt] = bt
                    for b in range(BLOC):
                        # --- dots + bias + exp over all j tiles ---
                        # tile_position'd matmul outputs must be PSUM
                        # bank-aligned (half-bank offsets crash the device),
                        # so each head gets a full 512-wide bank.
                        exp_t = {}
                        i0 = isl * 512
                        for jt in range(8):
                            et = expp.tile([128, 4, 512], BF16, tag="exp", name=f"exp{b}{jt}")
                            exp_t[jt] = et
                            dtiles = []
                            for pair in range(2):
                                dps = dpsum.tile([128, 2, 512], F32, tag="dps")
                                dtiles.append(dps)
                                for half in range(2):
                                    h = 2 * pair + half
                                    if h < 3:
                                        nc.tensor.matmul(
                                            dps[:, half, :],
                                            kt[b, hg][32 * h:32 * h + 32,
                                                      jt * 128:(jt + 1) * 128],
                                            qt[b, hg][32 * h:32 * h + 32,
                                                      i0:i0 + 512],
                                            start=True, stop=False,
                                            tile_position=(32 * h, 0),
                                        )
                                    else:
                                        # head 3: K=64 at row group 2 with
                                        # head-2 weight rows zeroed
                                        # (tile_position (96,0) is broken)
                                        nc.tensor.matmul(
                                            dps[:, half, :],
                                            kzt[b, hg][64:128,
                                                       jt * 128:(jt + 1) * 128],
                                            qt[b, hg][64:128, i0:i0 + 512],
                                            start=True, stop=False,
                                            tile_position=(64, 0),
                                        )
                            # all bias matmuls together: identity stationary
                            # loads once per j tile
                            for pair in range(2):
                                for half in range(2):
                                    h = 2 * pair + half
                                    nc.tensor.matmul(
                                        dtiles[pair][:, half, :],
                                        ident_s[:],
                                        bias_t[jt][:, h, :],
                                        start=False, stop=True,
                                    )
                            for pair in range(2):
                                ae = nc.scalar.activation(
                                    et[:, 2 * pair:2 * pair + 2, :],
                                    dtiles[pair][:], Exp, scale=SCALE)
                                last_exp[0] = ae
                        # --- AV + softmax sums, head pairs in col groups ---
                        for p in range(2):
                            av = avpsum.tile([128, 512], F32, tag="av")
                            sums = avpsum.tile([128, 512], F32, tag="sums")
                            for half in range(2):
                                h = 2 * p + half
                                hglob = 4 * hg + h
                                for jt in range(8):
                                    nc.tensor.matmul(
                                        av[64 * half:64 * half + 64, :],
                                        vt[b, jt][:, hglob, :],
                                        exp_t[jt][:, h, :],
                                        start=(jt == 0), stop=(jt == 7),
                                        tile_position=(0, 64 * half),
                                    )
                            # ones-stationary sums after av: ones load once,
                            # replicated sum lands on the same partitions as av
                            for half in range(2):
                                h = 2 * p + half
                                for jt in range(8):
                                    nc.tensor.matmul(
                                        sums[64 * half:64 * half + 64, :],
                                        ones_s[:],
                                        exp_t[jt][:, h, :],
                                        start=(jt == 0), stop=(jt == 7),
                                        tile_position=(0, 64 * half),
                                    )
                            rec = recp.tile([128, 512], F32, tag="rec")
                            nc.vector.reciprocal(rec[:], sums[:])
                            dt = 2 * hg + p
                            nc.vector.tensor_mul(
                                gelu_t[b, dt][:, isl * 512:(isl + 1) * 512],
                                av[:], rec[:])

            # ---------------- GELU + output projection ----------------
            for b in range(BLOC):
                for dt in range(4):
                    gi = nc.scalar.activation(gelu_t[b, dt][:], gelu_t[b, dt][:],
                                              Gelu, bias=ovg_s[:, dt:dt + 1],
                                              scale=1.0)
                    if last_exp[0] is not None:
                        add_dep_helper(gi.ins, last_exp[0].ins, sync=False,
                                       reason="group ACT table sets")
            for b in range(BLOC):
                for it in range(8):
                    ops = avpsum.tile([128, C], F32, tag="sums")
                    for dt in range(4):
                        nc.tensor.matmul(
                            ops[:],
                            gelu_t[b, dt][:, it * 128:(it + 1) * 128],
                            wo_s[:, dt, :],
                            start=(dt == 0), stop=(dt == 3),
                        )
                    osb = outp.tile([128, C], F32, tag="osb")
                    nc.vector.tensor_add(osb[:], ops[:], bout_s[:])
                    dma.dma_start(out_d[b, it * 128:(it + 1) * 128, :], osb[:])

    nc.compile()
    return nc


def _host_prep(x, w_q, bn_q, w_k, bn_k, w_v, bn_v, w_out, b_out, bn_out,
               pos_table):
    """Fold BN into weights, build bias table, shard across cores."""
    def fold(bn):
        g, b_, m, v = [np.asarray(a, np.float64) for a in bn]
        s = g / np.sqrt(v + EPS)
        return s, b_ - m * s

    sq, oq = fold(bn_q)
    sk, ok = fold(bn_k)
    sv, ov = fold(bn_v)
    so, oo = fold(bn_out)

    def wtile(w, s, ncols):
        # [C_in, D] * s[D] -> [128, C_in//128, D] bf16 (partition-major)
        w_eff = (np.asarray(w, np.float64) * s[None, :]).astype(np.float32)
        return np.ascontiguousarray(
            w_eff.reshape(-1, 128, ncols).transpose(1, 0, 2)).astype(NPBF16)

    wq = wtile(w_q, sq, C)
    wk = wtile(w_k, sk, C)
    wv = wtile(w_v, sv, IDV)
    wo = wtile(w_out, so, C)

    oq_t = np.ascontiguousarray(oq.astype(np.float32).reshape(2, 128).T)
    ok_t = np.ascontiguousarray(ok.astype(np.float32).reshape(2, 128).T)
    ovg_t = np.ascontiguousarray(ov.astype(np.float32).reshape(4, 128).T)
    bout_eff = (np.asarray(b_out, np.float64) * so + oo).astype(np.float32)
    bout_t = np.ascontiguousarray(np.broadcast_to(bout_eff, (128, C)))

    # position bias table
    r = np.arange(32)
    pos = np.stack(np.meshgrid(r, r, indexing="ij"), axis=-1).reshape(-1, 2)
    rel = np.abs(pos[:, None, :] - pos[None, :, :])
    idx = rel[..., 0] * 32 + rel[..., 1]           # [n, n]
    bias = np.asarray(pos_table, np.float32)[idx]  # [j, i, 8]
    bias = bias / (SCALE * SCALE)
    # -> [hg, is, jt, j1, h, i1]
    bias = bias.reshape(8, 128, 2, 512, 2, 4)      # jt, j1, is, i1, hg, h
    bias = np.ascontiguousarray(
        bias.transpose(4, 2, 0, 1, 5, 3)).astype(NPBF16)

    ident = np.eye(128, dtype=NPBF16)

    x = np.asarray(x, np.float32).reshape(-1, N, C)      # [B, n, C]
    common = dict(wq=wq, wk=wk, wv=wv, wo=wo, oq=oq_t, ok=ok_t, ovg=ovg_t,
                  bout=bout_t, bias=bias, ident=ident)
    in_maps = []
    for c in range(NCORES):
        xl = x[c * BLOC:(c + 1) * BLOC]                  # [2, n, C]
        xtl = xl.transpose(0, 2, 1).reshape(BLOC, 2, 128, N).astype(NPBF16)
        in_maps.append(dict(common, xt=np.ascontiguousarray(xtl)))
    return in_maps


def kernel(**inputs):
    if "nc" not in _CACHE:
        _CACHE["nc"] = _build_nc()
    nc = _CACHE["nc"]
    in_maps = _host_prep(**inputs)
    res = run_bass_kernel_spmd(nc, in_maps, core_ids=list(range(NCORES)),
                               trace=bool(int(os.environ.get("KTRACE", "0"))))
    _CACHE["last_result"] = res
    outs = [res.results[c]["out"].reshape(BLOC, 32, 32, C)
            for c in range(NCORES)]
    return np.concatenate(outs, axis=0).astype(np.float32)


if __name__ == "__main__":
    nc = _build_nc()
    print("build + compile OK")



# revision 2
# speedup vs baseline: 1.0553x; 1.0553x over previous
"""Trainium2 Bass kernel: 8-head attention block (BN-folded projections,
relative-position bias, softmax, GELU + output projection).

Sharding: data-parallel over batch across 8 NeuronCores (2 batch elems/core).
All weights / bias tables replicated; no collectives.

v2 redesign targeted at the TimelineSim cost model (matmul cost = out free
size x 1 cycle @2.4GHz for bf16; DVE tensor ops = free size x 1.04ns, 0.5x
when all operands are packed bf16 in SBUF; ACT = free size x 0.83ns):

  dotsT[j, i] psum f32 = sum_d KT[d,j] QT[d,i]   (4 heads packed via PE row
                                                  groups, as before)
  exp = ScalarE Exp(scale * psum) -> bf16         (no bias add on PE!)
  bias applied MULTIPLICATIVELY post-exp on DVE:  E *= exp(bias/scale)
                                                  (bf16 in-place mul, 2x mode)
  AV TRANSPOSED: T[i, (d|sum)] = sum_j E[j,i] * V'[j, (d|1)] where V' has a
  ones column appended -> softmax denominator lands as a per-partition column
  of the AV output, so normalisation is one reciprocal [128,4] plus one
  broadcast multiply per (b, head, i-half) on DVE. The V BN offset ov is
  pre-added into V' (exact: (sum E (v+ov))/s = out/s + ov), so GELU needs no
  per-partition bias.
  gelu input is [i, (h d)]; out proj needs [(h d), i] -> PE transpose-mode
  (128x128 blocks into psum, DVE copy back), then the usual out projection.

  HW workarounds kept from v1 (device crashes otherwise):
  - tile_position (96,0) is fatal (quadrant-3 bug): head 3 runs as a K=64
    matmul at (64,0) against a KT copy with head-2 rows zeroed.
  - tile_position'd matmuls keep PSUM-bank-aligned outputs.
  - exp and gelu forced into disjoint phases so the ScalarE activation
    table loads exactly twice.
"""

import os
import numpy as np
import ml_dtypes

import concourse.bass as bass
import concourse.tile as tile
from concourse import bacc, mybir
from concourse.bass import AP
from concourse.bass_utils import run_bass_kernel_spmd
from concourse.tile import add_dep_helper

NPBF16 = ml_dtypes.bfloat16
BF16 = mybir.dt.bfloat16
F32 = mybir.dt.float32

HEADS, DK, DV = 8, 32, 64
N = 1024          # positions = 32*32
C = 256           # channels
IDV = HEADS * DV  # 512
NCORES = 8
BLOC = 2          # batch elems per core
SCALE = float(DK) ** -0.5
EPS = 1e-5

_CACHE = {}


def _build_nc():
    nc = bacc.Bacc("TRN2", target_bir_lowering=False, debug=False)

    xt_d = nc.declare_dram_parameter("xt", [BLOC, 2, 128, N], BF16, isOutput=False)
    wq_d = nc.declare_dram_parameter("wq", [128, 2, C], BF16, isOutput=False)
    wk_d = nc.declare_dram_parameter("wk", [128, 2, C], BF16, isOutput=False)
    wv_d = nc.declare_dram_parameter("wv", [128, 2, IDV], BF16, isOutput=False)
    wo_d = nc.declare_dram_parameter("wo", [128, 4, C], BF16, isOutput=False)
    oq_d = nc.declare_dram_parameter("oq", [128, 2], F32, isOutput=False)
    ok_d = nc.declare_dram_parameter("ok", [128, 2], F32, isOutput=False)
    ovb_d = nc.declare_dram_parameter("ovb", [128, IDV], F32, isOutput=False)
    bout_d = nc.declare_dram_parameter("bout", [1, C], BF16, isOutput=False)
    # ebias[hg, is, jt, j1, h, i1] = exp(pos_bias[j, i, 4*hg+h] / SCALE)
    ebias_d = nc.declare_dram_parameter("ebias", [2, 2, 8, 128, 4, 512], BF16,
                                        isOutput=False)
    id_d = nc.declare_dram_parameter("ident", [128, 128], BF16, isOutput=False)
    out_d = nc.declare_dram_parameter("out", [BLOC, N, C], F32, isOutput=True)

    Exp = mybir.ActivationFunctionType.Exp
    Gelu = mybir.ActivationFunctionType.Gelu

    with tile.TileContext(nc) as tc:
        with (
            tc.tile_pool(name="const", bufs=1) as const,
            tc.tile_pool(name="persist", bufs=1) as persist,
            tc.tile_pool(name="biasp", bufs=8) as biasp,
            tc.tile_pool(name="expp", bufs=20) as expp,
            tc.tile_pool(name="recp", bufs=4) as recp,
            tc.tile_pool(name="outp", bufs=4) as outp,
            tc.tile_pool(name="dpsum", bufs=3, space="PSUM") as dpsum,
            tc.tile_pool(name="mpsum", bufs=2, space="PSUM") as mpsum,
        ):
            dma = nc.sync

            # ---------------- constants ----------------
            # first unit's inputs lead the SP/HWDGE queue; batch-1 x tiles
            # go through the Pool/SWDGE queue in parallel
            xt = {}
            for b in range(BLOC):
                for ct in range(2):
                    xt[b, ct] = persist.tile([128, N], BF16, tag=f"xt{b}{ct}",
                                             name=f"xt{b}{ct}")
            dma.dma_start(xt[0, 0][:], xt_d[0, 0])
            wq_s = const.tile([128, 2, C], BF16, tag="wq")
            dma.dma_start(wq_s[:], wq_d[:])
            oq_s = const.tile([128, 2], F32, tag="oq")
            dma.dma_start(oq_s[:], oq_d[:])
            dma.dma_start(xt[0, 1][:], xt_d[0, 1])
            wk_s = const.tile([128, 2, C], BF16, tag="wk")
            dma.dma_start(wk_s[:], wk_d[:])
            ok_s = const.tile([128, 2], F32, tag="ok")
            dma.dma_start(ok_s[:], ok_d[:])
            for ct in range(2):
                nc.gpsimd.dma_start(xt[1, ct][:], xt_d[1, ct])
            wv_s = const.tile([128, 2, IDV], BF16, tag="wv")
            dma.dma_start(wv_s[:], wv_d[:])
            ovb_s = const.tile([128, IDV], F32, tag="ovb")
            dma.dma_start(ovb_s[:], ovb_d[:])
            wo_s = const.tile([128, 4, C], BF16, tag="wo")
            dma.dma_start(wo_s[:], wo_d[:])
            bout_s = const.tile([1, C], BF16, tag="bout")
            dma.dma_start(bout_s[:], bout_d[:])
            ones1_s = const.tile([1, 128], BF16, tag="ones1")
            nc.vector.memset(ones1_s[:], 1.0)
            ident_s = const.tile([128, 128], BF16, tag="ident")
            dma.dma_start(ident_s[:], id_d[:])

            # dummy matmuls ramp the PE clock out of its cold p-state and
            # bridge the gap until the first input DMAs land
            warm = const.tile([1, 512], BF16, tag="warm")
            nc.gpsimd.memset(warm[:], 0.0)
            wps = dpsum.tile([128, 2, 512], F32, tag="dps")
            for w in range(2):
                nc.tensor.matmul(wps[:, 0, :], warm[:, 0:128], warm[:],
                                 start=(w == 0), stop=(w == 1))

            # ---------------- Q/K projections -> QT/KT [d, i] bf16 ----------
            # Emission order matters: the tile scheduler roughly follows
            # program order per engine, so emit only what the first dots
            # tiles need (hg=0 Q/K) first, then V / hg=1 while exp runs.
            qt, kt, kzt = {}, {}, {}

            def qk_proj(b, tt):
                qtile = persist.tile([128, N], BF16, tag=f"qt{b}{tt}", name=f"qt{b}{tt}")
                ktile = persist.tile([128, N], BF16, tag=f"kt{b}{tt}", name=f"kt{b}{tt}")
                qt[b, tt], kt[b, tt] = qtile, ktile
                # kz: copy of KT with head-2 rows zeroed; lets head 3 run
                # as a K=64 matmul at row group 2 (tile_position (96,0)
                # crashes this runtime: quadrant-3 HW bug)
                kz = persist.tile([128, N], BF16, tag=f"kz{b}{tt}",
                                  name=f"kz{b}{tt}")
                kzt[b, tt] = kz
                nc.gpsimd.memset(kz[64:96, :], 0.0)
                for wsb, osb, dst in ((wq_s, oq_s, qtile), (wk_s, ok_s, ktile)):
                    for i2 in range(2):
                        ps = dpsum.tile([128, 2, 512], F32, tag="dps")
                        for ct in range(2):
                            nc.tensor.matmul(
                                ps[:, 0, :],
                                wsb[:, ct, tt * 128:(tt + 1) * 128],
                                xt[b, ct][:, i2 * 512:(i2 + 1) * 512],
                                start=(ct == 0), stop=(ct == 1),
                            )
                        nc.vector.tensor_scalar_add(
                            dst[:, i2 * 512:(i2 + 1) * 512], ps[:, 0, :],
                            osb[:, tt:tt + 1])
                        if dst is ktile:
                            nc.vector.tensor_scalar_add(
                                kz[96:128, i2 * 512:(i2 + 1) * 512],
                                ps[96:128, 0, :], osb[96:128, tt:tt + 1])

            # --- V projection -> V' [j, h, 65] bf16 (ov added, ones col) ---
            vt = {}

            def v_proj(b, it):
                v = persist.tile([128, 8, 65], BF16, tag=f"v{b}{it}", name=f"v{b}{it}")
                vt[b, it] = v
                ps = dpsum.tile([128, 2, 512], F32, tag="dps")
                for ct in range(2):
                    nc.tensor.matmul(
                        ps[:, 0, :],
                        xt[b, ct][:, it * 128:(it + 1) * 128],
                        wv_s[:, ct, :],
                        start=(ct == 0), stop=(ct == 1),
                    )
                nc.vector.tensor_tensor(
                    v[:, :, 0:64],
                    ps[:, 0, :].rearrange("p (h d) -> p h d", h=8),
                    ovb_s[:].rearrange("p (h d) -> p h d", h=8),
                    mybir.AluOpType.add)
                nc.gpsimd.memset(v[:, :, 64:65], 1.0)

            for b in range(BLOC):
                qk_proj(b, 0)

            # gelu input tiles [128 = i slice, 8 it, 512 = (h d)] bf16, per b
            gelu_t = {}
            for b in range(BLOC):
                gelu_t[b] = persist.tile([128, 8, 512], BF16, tag=f"g{b}", name=f"g{b}")
            gt = {}
            for b in range(BLOC):
                gt[b] = persist.tile([128, 4, N], BF16, tag=f"gt{b}",
                                     name=f"gt{b}")

            # ---------------- attention (software-pipelined) ----------------
            bias_cache = {}

            def get_bias(hg, isl):
                if (hg, isl) not in bias_cache:
                    bias_t = {}
                    for jt in range(8):
                        bt = biasp.tile([128, 4, 512], BF16, tag="bias",
                                        name=f"bias{hg}{isl}{jt}")
                        dma.dma_start(bt[:], ebias_d[hg, isl, jt])
                        bias_t[jt] = bt
                    bias_cache[hg, isl] = bias_t
                return bias_cache[hg, isl]

            def emit_dots_exp(hg, isl, b, extras=()):
                """dots -> exp -> *ebias for one (hg, isl, b) unit.
                `extras` are deferred emission thunks (projection work)
                interleaved into the per-jt stream so their DVE ops don't
                pile up behind a full unit of bias-muls on the in-order
                DVE queue. Returns the exp tiles for the deferred AV."""
                extras = list(extras)  # (min_jt, thunk), emitted in order
                bias_t = get_bias(hg, isl)
                exp_all = {}
                i0 = isl * 512
                if True:
                    for jt in range(8):
                        et = expp.tile([128, 4, 512], BF16, tag="exp",
                                       name=f"exp{hg}{isl}{b}{jt}")
                        exp_all[b, jt] = et
                        for pair in range(2):
                            dps = dpsum.tile([128, 2, 512], F32, tag="dps")
                            for half in range(2):
                                h = 2 * pair + half
                                if h < 3:
                                    nc.tensor.matmul(
                                        dps[:, half, :],
                                        kt[b, hg][32 * h:32 * h + 32,
                                                  jt * 128:(jt + 1) * 128],
                                        qt[b, hg][32 * h:32 * h + 32,
                                                  i0:i0 + 512],
                                        start=True, stop=True,
                                        tile_position=(32 * h, 0),
                                    )
                                else:
                                    # head 3: K=64 at row group 2 with
                                    # head-2 weight rows zeroed
                                    # (tile_position (96,0) is broken)
                                    nc.tensor.matmul(
                                        dps[:, half, :],
                                        kzt[b, hg][64:128,
                                                   jt * 128:(jt + 1) * 128],
                                        qt[b, hg][64:128, i0:i0 + 512],
                                        start=True, stop=True,
                                        tile_position=(64, 0),
                                    )
                            nc.scalar.activation(
                                et[:, 2 * pair:2 * pair + 2, :],
                                dps[:], Exp, scale=SCALE)
                        # multiplicative position bias; two of eight tiles
                        # go to the otherwise-idle GpSimd engine
                        eng = nc.gpsimd if jt == 3 else nc.vector
                        eng.tensor_tensor(
                            et[:], et[:], bias_t[jt][:],
                            mybir.AluOpType.mult)
                        budget = 1
                        while budget and extras and extras[0][0] <= jt:
                            extras.pop(0)[1]()
                            budget -= 1
                for _, th in extras:
                    th()
                return exp_all

            def emit_av(hg, isl, b, exp_all, heads=range(4)):
                """transposed AV + softmax normalisation for one unit."""
                if True:
                    for h in heads:
                        hglob = 4 * hg + h
                        T4 = mpsum.tile([128, 4, 128], F32, tag="m")
                        for itl in range(4):
                            for jt in range(8):
                                nc.tensor.matmul(
                                    T4[:, itl, 0:65],
                                    exp_all[b, jt][:, h, itl * 128:(itl + 1) * 128],
                                    vt[b, jt][:, hglob, :],
                                    start=(itl == 0 and jt == 0),
                                    stop=(itl == 3 and jt == 7),
                                )
                        r4 = recp.tile([128, 4], F32, tag="r4")
                        nc.vector.reciprocal(
                            r4[:], T4[:, :, 64:65].rearrange("p a b -> p (a b)"))
                        r4b = AP(r4[:].tensor, r4[:].offset,
                                 [r4[:].ap[0], r4[:].ap[1], [0, 64]])
                        nc.vector.tensor_tensor(
                            gelu_t[b][:, isl * 4:isl * 4 + 4,
                                      hglob * 64:(hglob + 1) * 64],
                            T4[:, :, 0:64], r4b, mybir.AluOpType.mult)

            def tail_piece(isl, b, itl, on_act, pool=None):
                """transpose + out-projection + DMA for one (b, i-tile);
                psum->sbuf copies on ScalarE when on_act (tail-end only).
                `pool` picks the psum pool: the final tail borrows the dots
                pool (dots are done) so two pieces can be in flight."""
                pool = pool or mpsum
                def cp(out, in_):
                    if on_act:
                        nc.scalar.copy(out, in_)
                    else:
                        nc.vector.tensor_copy(out, in_)
                it = isl * 4 + itl
                tp = pool.tile([128, 4, 128], BF16, tag="m" if pool is mpsum else "dps")
                for dblk in range(4):
                    nc.tensor.transpose(
                        tp[:, dblk, :],
                        gelu_t[b][:, it, dblk * 128:(dblk + 1) * 128],
                        ident_s[:])
                cp(gt[b][:, :, it * 128:(it + 1) * 128], tp[:])
                ops = pool.tile([128, 4, 128], F32, tag="m" if pool is mpsum else "dps")
                for dblk in range(4):
                    nc.tensor.matmul(
                        ops[:].rearrange("p a b -> p (a b)")[:, 0:C],
                        gt[b][:, dblk, it * 128:(it + 1) * 128],
                        wo_s[:, dblk, :],
                        start=(dblk == 0), stop=False,
                    )
                # K=1 ones x bout row folds the output bias into the
                # matmul accumulation (frees DVE of a broadcast add)
                nc.tensor.matmul(
                    ops[:].rearrange("p a b -> p (a b)")[:, 0:C],
                    ones1_s[:], bout_s[:],
                    start=False, stop=True,
                )
                osb = outp.tile([128, C], F32, tag="osb")
                cp(osb[:], ops[:].rearrange("p a b -> p (a b)")[:, 0:C])
                dma.dma_start(
                    out_d[b, it * 128:(it + 1) * 128, :], osb[:])

            def emit_tail(isl, last=False):
                """GELU + interleaved per-(b, i-tile) output chains; with
                `last`, b=1's copies ride the then-idle ScalarE so both
                batch elements' chains pipeline against PE."""
                for b in range(BLOC):
                    nc.scalar.activation(
                        gelu_t[b][:, isl * 4:isl * 4 + 4, :],
                        gelu_t[b][:, isl * 4:isl * 4 + 4, :], Gelu, scale=1.0)
                for itl in range(4):
                    tail_piece(isl, 0, itl, on_act=False)
                    tail_piece(isl, 1, itl, on_act=last,
                               pool=dpsum if last else None)

            # pipeline: dots/exp of unit k+1 is emitted before AV of unit k,
            # so PE always has dots to chew while ACT works through exps.
            # units are (isl, hg, b); isl-major so the output tail for each
            # i-half can overlap the other half's attention.
            units = [(isl, hg, b)
                     for isl in range(2) for hg in range(2) for b in range(BLOC)]
            extras_a = [(1, lambda: qk_proj(0, 1))]
            extras_a += [(1, lambda it=it: v_proj(0, it)) for it in range(8)]
            extras_b = [(1, lambda: qk_proj(1, 1))]
            extras_b += [(1, lambda it=it: v_proj(1, it)) for it in range(8)]
            pending = None  # (hg, isl, b, exp tiles)
            for k, (isl, hg, b) in enumerate(units):
                extras = []
                if k == 1:
                    extras += extras_a
                elif k == 2:
                    extras += extras_b
                if pending is not None and k >= 2:
                    # AV of unit k-1 threads into this unit's back half so
                    # it never delays the next unit's dots
                    ph, pi, pb, pe_ = pending
                    base = 5 if k == 2 else 4
                    extras += [(base + h,
                                lambda h=h: emit_av(ph, pi, pb, pe_,
                                                    heads=(h,)))
                               for h in range(4)]
                    pending = None
                e = emit_dots_exp(hg, isl, b, extras=extras)
                if pending is not None:
                    emit_av(*pending)  # only AV of unit 0 takes this path
                pending = (hg, isl, b, e)
                if k == 4:
                    # all isl=0 AV has been emitted; its output tail can
                    # overlap the isl=1 attention work
                    emit_tail(0)
            emit_av(*pending)
            emit_tail(1, last=True)

    nc.compile()
    return nc


def _host_prep(x, w_q, bn_q, w_k, bn_k, w_v, bn_v, w_out, b_out, bn_out,
               pos_table):
    """Fold BN into weights, build multiplicative bias table, shard."""
    def fold(bn):
        g, b_, m, v = [np.asarray(a, np.float64) for a in bn]
        s = g / np.sqrt(v + EPS)
        return s, b_ - m * s

    sq, oq = fold(bn_q)
    sk, ok = fold(bn_k)
    sv, ov = fold(bn_v)
    so, oo = fold(bn_out)

    def wtile(w, s, ncols):
        # [C_in, D] * s[D] -> [128, C_in//128, D] bf16 (partition-major)
        w_eff = (np.asarray(w, np.float64) * s[None, :]).astype(np.float32)
        return np.ascontiguousarray(
            w_eff.reshape(-1, 128, ncols).transpose(1, 0, 2)).astype(NPBF16)

    wq = wtile(w_q, sq, C)
    wk = wtile(w_k, sk, C)
    wv = wtile(w_v, sv, IDV)
    wo = wtile(w_out, so, C)

    oq_t = np.ascontiguousarray(oq.astype(np.float32).reshape(2, 128).T)
    ok_t = np.ascontiguousarray(ok.astype(np.float32).reshape(2, 128).T)
    ovb_t = np.ascontiguousarray(
        np.broadcast_to(ov.astype(np.float32), (128, IDV)))
    bout_eff = (np.asarray(b_out, np.float64) * so + oo).astype(np.float32)
    bout_t = np.ascontiguousarray(bout_eff.reshape(1, C)).astype(NPBF16)

    # multiplicative position bias table: exp(bias / SCALE)
    r = np.arange(32)
    pos = np.stack(np.meshgrid(r, r, indexing="ij"), axis=-1).reshape(-1, 2)
    rel = np.abs(pos[:, None, :] - pos[None, :, :])
    idx = rel[..., 0] * 32 + rel[..., 1]           # [n, n]
    bias = np.asarray(pos_table, np.float64)[idx]  # [j, i, 8]
    ebias = np.exp(bias / SCALE).astype(np.float32)
    # -> [hg, is, jt, j1, h, i1]
    ebias = ebias.reshape(8, 128, 2, 512, 2, 4)    # jt, j1, is, i1, hg, h
    ebias = np.ascontiguousarray(
        ebias.transpose(4, 2, 0, 1, 5, 3)).astype(NPBF16)

    ident = np.eye(128, dtype=NPBF16)

    x = np.asarray(x, np.float32).reshape(-1, N, C)      # [B, n, C]
    common = dict(wq=wq, wk=wk, wv=wv, wo=wo, oq=oq_t, ok=ok_t, ovb=ovb_t,
                  bout=bout_t, ebias=ebias, ident=ident)
    in_maps = []
    for c in range(NCORES):
        xl = x[c * BLOC:(c + 1) * BLOC]                  # [2, n, C]
        xtl = xl.transpose(0, 2, 1).reshape(BLOC, 2, 128, N).astype(NPBF16)
        in_maps.append(dict(common, xt=np.ascontiguousarray(xtl)))
    return in_maps


def kernel(**inputs):
    if "nc" not in _CACHE:
        _CACHE["nc"] = _build_nc()
    nc = _CACHE["nc"]
    in_maps = _host_prep(**inputs)
    res = run_bass_kernel_spmd(nc, in_maps, core_ids=list(range(NCORES)),
                               trace=bool(int(os.environ.get("KTRACE", "0"))))
    _CACHE["last_result"] = res
    outs = [res.results[c]["out"].reshape(BLOC, 32, 32, C)
            for c in range(NCORES)]
    return np.concatenate(outs, axis=0).astype(np.float32)


if __name__ == "__main__":
    nc = _build_nc()
    print("build + compile OK")
